# revision 7
# baseline (speedup 1.0000x reference)
# BERT-base (12-layer) forward on 8 Trainium2 NeuronCores.
# Sharding: pure data-parallel over batch (B=8 -> 1 sequence per core, no
# collectives). Per core the full encoder stack runs with activations kept
# in transposed layout [H, tokens] so every projection consumes weights in
# their natural [in, out] layout as the PE stationary operand. Matmuls run
# as float32r (single-pass fp32, full PE rate at free-dim >= 256).
import numpy as np

import concourse.bass as bass
import concourse.mybir as mybir
import concourse.tile as tile
from concourse import bacc
from concourse.bass_utils import run_bass_kernel_spmd
from concourse.masks import make_identity

B, S, H, NH, HD = 8, 512, 768, 12, 64
FF, V, T = 3072, 30522, 2
HC = H // 128        # 6 chunks of H
FC = FF // 128       # 24 chunks of FF
TCH = S // 128       # 4 token chunks
EPS = 1e-12
F32 = mybir.dt.float32
F32R = mybir.dt.float32r
I32 = mybir.dt.int32
AF = mybir.ActivationFunctionType
OP = mybir.AluOpType
N_CORES = 8


def build_bert(L=12):
    nc = bacc.Bacc("TRN2", target_bir_lowering=False, debug=False,
                   num_devices=N_CORES)

    # ---- DRAM parameters (per-core; weights identical across cores) ----
    ids_d = nc.declare_dram_parameter("ids", [S], I32, isOutput=False)
    tts_d = nc.declare_dram_parameter("tts", [S], I32, isOutput=False)
    wemb_d = nc.declare_dram_parameter("wemb", [V, H], F32, isOutput=False)
    temb_d = nc.declare_dram_parameter("temb", [T, H], F32, isOutput=False)
    pemb_d = nc.declare_dram_parameter("pemb", [S, H], F32, isOutput=False)
    # stationary-block layouts: [L, m_chunk, 128(k_part), k_chunk, 128(m)]
    wq_d = nc.declare_dram_parameter("wq", [L, HC, 128, HC, 128], F32R, isOutput=False)
    wk_d = nc.declare_dram_parameter("wk", [L, HC, 128, HC, 128], F32R, isOutput=False)
    wo_d = nc.declare_dram_parameter("wo", [L, HC, 128, HC, 128], F32R, isOutput=False)
    wi_d = nc.declare_dram_parameter("wi", [L, FC, 128, HC, 128], F32R, isOutput=False)
    # moving-panel layouts (natural order)
    wv_d = nc.declare_dram_parameter("wv", [L, H, H], F32R, isOutput=False)
    wout_d = nc.declare_dram_parameter("wout", [L, FF, H], F32R, isOutput=False)
    pw_d = nc.declare_dram_parameter("pw", [HC, 128, HC, 128], F32R, isOutput=False)
    # biases, [128, chunks] per layer (transposed-layout partition order)
    bq_d = nc.declare_dram_parameter("bq", [L, 128, HC], F32, isOutput=False)
    bk_d = nc.declare_dram_parameter("bk", [L, 128, HC], F32, isOutput=False)
    bo_d = nc.declare_dram_parameter("bo", [L, 128, HC], F32, isOutput=False)
    bi_d = nc.declare_dram_parameter("bi", [L, 128, FC], F32, isOutput=False)
    bout_d = nc.declare_dram_parameter("bout", [L, 128, HC], F32, isOutput=False)
    pb_d = nc.declare_dram_parameter("pb", [128, HC], F32, isOutput=False)

    x_out_d = nc.declare_dram_parameter("x_out", [S, H], F32, isOutput=True)
    pool_out_d = nc.declare_dram_parameter("pool_out", [H], F32, isOutput=True)

    with tile.TileContext(nc) as tc:
        import contextlib
        ctx = contextlib.ExitStack()
        with ctx:
            sb = ctx.enter_context(tc.tile_pool(name="sb", bufs=1))
            acts = ctx.enter_context(tc.tile_pool(name="acts", bufs=1))
            xpool = ctx.enter_context(tc.tile_pool(name="xpool", bufs=2))
            vpool = ctx.enter_context(tc.tile_pool(name="vpool", bufs=4))
            expp = ctx.enter_context(tc.tile_pool(name="expp", bufs=5))
            interp = ctx.enter_context(tc.tile_pool(name="interp", bufs=5))
            x2p = ctx.enter_context(tc.tile_pool(name="x2p", bufs=2))
            wcolp = ctx.enter_context(tc.tile_pool(name="wcolp", bufs=5))
            wpanp = ctx.enter_context(tc.tile_pool(name="wpanp", bufs=7))
            biasp = ctx.enter_context(tc.tile_pool(name="biasp", bufs=2))
            rowp = ctx.enter_context(tc.tile_pool(name="rowp", bufs=3))
            rowr = ctx.enter_context(tc.tile_pool(name="rowr", bufs=2))
            sbbp = ctx.enter_context(tc.tile_pool(name="sbbp", bufs=2))
            tinyp = ctx.enter_context(tc.tile_pool(name="tinyp", bufs=2))
            gp = ctx.enter_context(tc.tile_pool(name="gp", bufs=1))
            ps_work = ctx.enter_context(tc.tile_pool(name="ps_work", bufs=3, space="PSUM"))
            ps_ctx = ctx.enter_context(tc.tile_pool(name="ps_ctx", bufs=2, space="PSUM"))
            ps_f2 = ctx.enter_context(tc.tile_pool(name="ps_f2", bufs=2, space="PSUM"))

            # ---- constants ----
            ident = sb.tile([128, 128], F32, tag="ident")
            make_identity(nc, ident[:])
            ones_col = sb.tile([128, 1], F32, tag="ones_col")   # stats lhsT
            nc.vector.memset(ones_col[:], 1.0)
            ones_row = sb.tile([1, 128], F32, tag="ones_row")   # bcast lhsT
            nc.vector.memset(ones_row[:], 1.0)
            eps_t = sb.tile([128, 1], F32, tag="eps_t")
            nc.vector.memset(eps_t[:], EPS)

            def bcast(src_row, n_part):
                """PE-broadcast [1,S] f32r row -> PSUM [n_part, S]."""
                p = ps_work.tile([128, S], F32, tag="ps_work")
                nc.tensor.matmul(out=p[0:n_part, :],
                                 lhsT=ones_row[0:1, 0:n_part].bitcast(F32R),
                                 rhs=src_row, start=True, stop=True)
                return p

            # =====================  EMBEDDINGS  =====================
            idt = tinyp.tile([128, TCH], I32, tag="idt")
            ttt = tinyp.tile([128, TCH], I32, tag="ttt")
            nc.sync.dma_start(out=idt[:], in_=ids_d[:].rearrange("(c p) -> p c", p=128))
            nc.sync.dma_start(out=ttt[:], in_=tts_d[:].rearrange("(c p) -> p c", p=128))

            x = xpool.tile([128, HC, S], F32R, tag="x")
            for tcc in range(TCH):
                g1 = gp.tile([128, H], F32, tag="g1")
                g2 = gp.tile([128, H], F32, tag="g2")
                g3 = gp.tile([128, H], F32, tag="g3")
                nc.gpsimd.indirect_dma_start(
                    out=g1[:], out_offset=None, in_=wemb_d[:, :],
                    in_offset=bass.IndirectOffsetOnAxis(ap=idt[:, tcc:tcc + 1], axis=0))
                nc.gpsimd.indirect_dma_start(
                    out=g2[:], out_offset=None, in_=temb_d[:, :],
                    in_offset=bass.IndirectOffsetOnAxis(ap=ttt[:, tcc:tcc + 1], axis=0))
                nc.sync.dma_start(out=g3[:], in_=pemb_d[tcc * 128:(tcc + 1) * 128, :])
                nc.vector.tensor_add(out=g1[:], in0=g1[:], in1=g2[:])
                nc.vector.tensor_add(out=g1[:], in0=g1[:], in1=g3[:])
                # LayerNorm over free dim (H) in standard layout
                stats = tinyp.tile([128, 3, 6], F32, tag="stats")
                for sg in range(3):
                    nc.vector.bn_stats(out=stats[:, sg, :], in_=g1[:, sg * 256:(sg + 1) * 256])
                mv = tinyp.tile([128, 2], F32, tag="mv")
                nc.vector.bn_aggr(out=mv[:], in_=stats[:])
                sd = tinyp.tile([128, 1], F32, tag="sd")
                nc.scalar.activation(out=sd[:], in_=mv[:, 1:2], func=AF.Sqrt, bias=eps_t[:])
                rstd = tinyp.tile([128, 1], F32, tag="rstd")
                nc.vector.reciprocal(out=rstd[:], in_=sd[:])
                nc.vector.tensor_scalar(out=g1[:], in0=g1[:], scalar1=mv[:, 0:1],
                                        scalar2=rstd[:], op0=OP.subtract, op1=OP.mult)
                # transpose into x[:, j, tc*128:+128]
                for j in range(HC):
                    pt = ps_work.tile([128, S], F32, tag="ps_work")
                    nc.tensor.transpose(out=pt[:, 0:128], in_=g1[:, j * 128:(j + 1) * 128],
                                        identity=ident[:])
                    nc.scalar.activation(out=x[:, j, tcc * 128:(tcc + 1) * 128],
                                         in_=pt[:, 0:128], func=AF.Copy)

            # ---- LayerNorm over the partition (H) axis ----
            def emit_ln(src, dst):
                p_sum = ps_work.tile([128, S], F32, tag="ps_work")
                for jj in range(HC):
                    nc.tensor.matmul(out=p_sum[0:1, :],
                                     lhsT=ones_col[:, 0:1].bitcast(F32R),
                                     rhs=src[:, jj, :],
                                     start=(jj == 0), stop=(jj == HC - 1))
                p_sq = ps_work.tile([128, S], F32, tag="ps_work")
                for jj in range(HC):
                    sq = x2p.tile([128, S], F32R, tag="x2")
                    nc.scalar.activation(out=sq[:], in_=src[:, jj, :].bitcast(F32),
                                         func=AF.Square)
                    nc.tensor.matmul(out=p_sq[0:1, :],
                                     lhsT=ones_col[:, 0:1].bitcast(F32R),
                                     rhs=sq[:],
                                     start=(jj == 0), stop=(jj == HC - 1))
                mean = rowp.tile([1, S], F32, tag="lnrow")
                nc.vector.tensor_scalar_mul(out=mean[:], in0=p_sum[0:1, :],
                                            scalar1=1.0 / H)
                msq = rowp.tile([1, S], F32, tag="lnrow")
                nc.vector.tensor_mul(out=msq[:], in0=mean[:], in1=mean[:])
                var = rowp.tile([1, S], F32, tag="lnrow")
                nc.vector.scalar_tensor_tensor(out=var[:], in0=p_sq[0:1, :],
                                               scalar=1.0 / H, in1=msq[:],
                                               op0=OP.mult, op1=OP.subtract)
                sd2 = rowp.tile([1, S], F32, tag="lnrow")
                nc.scalar.activation(out=sd2[:], in_=var[:], func=AF.Sqrt,
                                     bias=eps_t[0:1, :])
                rs = rowr.tile([1, S], F32R, tag="lnrowr")
                with nc.allow_low_precision(reason="fp32r row feeds PE broadcast"):
                    nc.vector.reciprocal(out=rs[:], in_=sd2[:])
                mrs = rowr.tile([1, S], F32R, tag="lnrowr")
                nc.vector.tensor_mul(out=mrs[:], in0=mean[:], in1=rs[:].bitcast(F32))
                p_r = bcast(rs[:], 128)
                p_m = bcast(mrs[:], 128)
                for jj in range(HC):
                    nc.vector.tensor_mul(out=dst[:, jj, :],
                                         in0=src[:, jj, :].bitcast(F32), in1=p_r[:])
                    nc.vector.tensor_sub(out=dst[:, jj, :],
                                         in0=dst[:, jj, :].bitcast(F32), in1=p_m[:])

            # =====================  ENCODER LAYERS  =====================
            for l in range(L):
                bq_t = biasp.tile([128, HC], F32, tag="bq")
                bk_t = biasp.tile([128, HC], F32, tag="bk")
                bo_t = biasp.tile([128, HC], F32, tag="bo")
                bi_t = biasp.tile([128, FC], F32, tag="bi")
                bout_t = biasp.tile([128, HC], F32, tag="bout")
                nc.sync.dma_start(out=bq_t[:], in_=bq_d[l])
                nc.sync.dma_start(out=bk_t[:], in_=bk_d[l])
                nc.sync.dma_start(out=bo_t[:], in_=bo_d[l])
                nc.sync.dma_start(out=bi_t[:], in_=bi_d[l])
                nc.sync.dma_start(out=bout_t[:], in_=bout_d[l])

                # ---- Q/K projections (transposed layout out) ----
                q_t = acts.tile([128, HC, S], F32R, tag="q")
                k_t = acts.tile([128, HC, S], F32R, tag="k")
                for m in range(HC):
                    wq_m = wcolp.tile([128, HC, 128], F32R, tag="wcol")
                    nc.sync.dma_start(out=wq_m[:], in_=wq_d[l, m])
                    pq = ps_work.tile([128, S], F32, tag="ps_work")
                    for kc in range(HC):
                        nc.tensor.matmul(out=pq[:], lhsT=wq_m[:, kc, :], rhs=x[:, kc, :],
                                         start=(kc == 0), stop=(kc == HC - 1))
                    nc.vector.tensor_scalar_add(out=q_t[:, m, :], in0=pq[:],
                                                scalar1=bq_t[:, m:m + 1])
                    wk_m = wcolp.tile([128, HC, 128], F32R, tag="wcol")
                    nc.sync.dma_start(out=wk_m[:], in_=wk_d[l, m])
                    pk = ps_work.tile([128, S], F32, tag="ps_work")
                    for kc in range(HC):
                        nc.tensor.matmul(out=pk[:], lhsT=wk_m[:, kc, :], rhs=x[:, kc, :],
                                         start=(kc == 0), stop=(kc == HC - 1))
                    nc.vector.tensor_scalar_add(out=k_t[:, m, :], in0=pk[:],
                                                scalar1=bk_t[:, m:m + 1])

                # ---- V projection (standard layout, ones column appended) ----
                wv_p = []
                for kc in range(HC):
                    w = wpanp.tile([128, H], F32R, tag="wpan")
                    nc.sync.dma_start(out=w[:], in_=wv_d[l, kc * 128:(kc + 1) * 128, :])
                    wv_p.append(w)
                v_t = []
                for tcc in range(TCH):
                    v = vpool.tile([128, NH, HD + 1], F32R, tag="v")
                    nc.vector.memset(v[:, :, HD:HD + 1].bitcast(F32), 1.0)
                    for half in range(2):
                        pv = ps_work.tile([128, S], F32, tag="ps_work")
                        for kc in range(HC):
                            nc.tensor.matmul(
                                out=pv[:, 0:384],
                                lhsT=x[:, kc, tcc * 128:(tcc + 1) * 128],
                                rhs=wv_p[kc][:, half * 384:(half + 1) * 384],
                                start=(kc == 0), stop=(kc == HC - 1))
                        nc.vector.tensor_copy(
                            out=v[:, half * 6:(half + 1) * 6, 0:HD],
                            in_=pv[:, 0:384].rearrange("p (h d) -> p h d", d=HD))
                    v_t.append(v)

                # ---- attention per head ----
                ctx_t = acts.tile([128, HC, S], F32R, tag="ctx")
                for h in range(NH):
                    bp = (h % 2) * 64
                    j = h // 2
                    pc = ps_ctx.tile([HD + 1, S], F32, tag="ps_ctx")
                    for kc in range(TCH):
                        ps_s = ps_work.tile([128, S], F32, tag="ps_work")
                        nc.tensor.matmul(
                            out=ps_s[:],
                            lhsT=k_t[bp:bp + HD, j, kc * 128:(kc + 1) * 128],
                            rhs=q_t[bp:bp + HD, j, :],
                            start=True, stop=True)
                        e = expp.tile([128, S], F32R, tag="exp")
                        nc.scalar.activation(out=e[:], in_=ps_s[:], func=AF.Exp,
                                             scale=0.125)
                        nc.tensor.matmul(out=pc[:], lhsT=v_t[kc][:, h, :], rhs=e[:],
                                         start=(kc == 0), stop=(kc == TCH - 1))
                    rec = rowr.tile([1, S], F32R, tag="rec")
                    with nc.allow_low_precision(reason="fp32r row feeds PE broadcast"):
                        nc.vector.reciprocal(out=rec[:], in_=pc[HD:HD + 1, :])
                    pb = bcast(rec[:], HD)
                    sbb = sbbp.tile([HD, S], F32, tag="sbb")
                    nc.scalar.activation(out=sbb[:], in_=pb[0:HD, :], func=AF.Copy)
                    nc.vector.tensor_mul(out=ctx_t[bp:bp + HD, j, :],
                                         in0=pc[0:HD, :], in1=sbb[:])

                # ---- attention out projection + residual ----
                r1 = acts.tile([128, HC, S], F32R, tag="resid")
                for m in range(HC):
                    wo_m = wcolp.tile([128, HC, 128], F32R, tag="wcol")
                    nc.sync.dma_start(out=wo_m[:], in_=wo_d[l, m])
                    po = ps_work.tile([128, S], F32, tag="ps_work")
                    for kc in range(HC):
                        nc.tensor.matmul(out=po[:], lhsT=wo_m[:, kc, :],
                                         rhs=ctx_t[:, kc, :],
                                         start=(kc == 0), stop=(kc == HC - 1))
                    nc.vector.scalar_tensor_tensor(
                        out=r1[:, m, :], in0=po[:], scalar=bo_t[:, m:m + 1],
                        in1=x[:, m, :].bitcast(F32), op0=OP.add, op1=OP.add)

                aln = acts.tile([128, HC, S], F32R, tag="aln")
                emit_ln(r1, aln)

                # ---- FFN ----
                r2 = acts.tile([128, HC, S], F32R, tag="resid")
                NBLK, BLK = 6, 4
                for blk in range(NBLK):
                    inter_blk = []
                    for mi in range(BLK):
                        m = blk * BLK + mi
                        wi_m = wcolp.tile([128, HC, 128], F32R, tag="wcol")
                        nc.sync.dma_start(out=wi_m[:], in_=wi_d[l, m])
                        pf = ps_work.tile([128, S], F32, tag="ps_work")
                        for kc in range(HC):
                            nc.tensor.matmul(out=pf[:], lhsT=wi_m[:, kc, :],
                                             rhs=aln[:, kc, :],
                                             start=(kc == 0), stop=(kc == HC - 1))
                        it = interp.tile([128, S], F32R, tag="inter")
                        nc.scalar.activation(out=it[:], in_=pf[:], func=AF.Gelu,
                                             bias=bi_t[:, m:m + 1], scale=1.0)
                        inter_blk.append(it)
                    wr = []
                    for mi in range(BLK):
                        m = blk * BLK + mi
                        w = wpanp.tile([128, H], F32R, tag="wpan")
                        nc.sync.dma_start(out=w[:], in_=wout_d[l, m * 128:(m + 1) * 128, :])
                        wr.append(w)
                    for hh in range(HC):
                        p2 = ps_f2.tile([128, S], F32, tag="ps_f2")
                        for mi in range(BLK):
                            nc.tensor.matmul(out=p2[:],
                                             lhsT=wr[mi][:, hh * 128:(hh + 1) * 128],
                                             rhs=inter_blk[mi][:],
                                             start=(mi == 0), stop=(mi == BLK - 1))
                        if blk == 0:
                            nc.vector.tensor_copy(out=r2[:, hh, :], in_=p2[:])
                        elif blk < NBLK - 1:
                            nc.vector.tensor_add(out=r2[:, hh, :],
                                                 in0=r2[:, hh, :].bitcast(F32), in1=p2[:])
                        else:
                            nc.vector.tensor_add(out=r2[:, hh, :],
                                                 in0=r2[:, hh, :].bitcast(F32), in1=p2[:])
                            nc.vector.scalar_tensor_tensor(
                                out=r2[:, hh, :], in0=r2[:, hh, :].bitcast(F32),
                                scalar=bout_t[:, hh:hh + 1],
                                in1=aln[:, hh, :].bitcast(F32),
                                op0=OP.add, op1=OP.add)

                x_next = xpool.tile([128, HC, S], F32R, tag="x")
                emit_ln(r2, x_next)
                x = x_next

            # =====================  POOLER  =====================
            pb_t = biasp.tile([128, HC], F32, tag="pb")
            nc.sync.dma_start(out=pb_t[:], in_=pb_d[:, :])
            pool_sb = tinyp.tile([128, HC], F32, tag="pool_sb")
            for m in range(HC):
                pw_m = wcolp.tile([128, HC, 128], F32R, tag="wcol")
                nc.sync.dma_start(out=pw_m[:], in_=pw_d[m])
                pp = ps_work.tile([128, S], F32, tag="ps_work")
                for kc in range(HC):
                    nc.tensor.matmul(out=pp[:, 0:128], lhsT=pw_m[:, kc, :],
                                     rhs=x[:, kc, 0:128],
                                     start=(kc == 0), stop=(kc == HC - 1))
                nc.scalar.activation(out=pool_sb[:, m:m + 1], in_=pp[:, 0:1],
                                     func=AF.Tanh, bias=pb_t[:, m:m + 1], scale=1.0)
            nc.sync.dma_start(out=pool_out_d[:].rearrange("(c p) -> p c", p=128),
                              in_=pool_sb[:])

            # =====================  FINAL X OUT  =====================
            for tcc in range(TCH):
                xo = gp.tile([128, H], F32, tag="g1")
                for j in range(HC):
                    pt = ps_work.tile([128, S], F32, tag="ps_work")
                    nc.tensor.transpose(out=pt[:, 0:128],
                                        in_=x[:, j, tcc * 128:(tcc + 1) * 128].bitcast(F32),
                                        identity=ident[:])
                    nc.scalar.activation(out=xo[:, j * 128:(j + 1) * 128],
                                         in_=pt[:, 0:128], func=AF.Copy)
                nc.sync.dma_start(out=x_out_d[tcc * 128:(tcc + 1) * 128, :], in_=xo[:])

    nc.compile()
    return nc


_nc_cache = {}


def _get_nc(L=12):
    if L not in _nc_cache:
        _nc_cache[L] = build_bert(L)
    return _nc_cache[L]


def _prep_weights(inputs, L=12):
    """Host-side rearrangement into DMA-friendly layouts (fp32 numpy)."""
    def stat_blocks(w):
        # [in, out] -> [m_chunk, 128(k_part), k_chunk, 128(m)]
        i, o = w.shape
        return np.ascontiguousarray(
            w.reshape(i // 128, 128, o // 128, 128).transpose(2, 1, 0, 3))

    d = {}
    d["wq"] = np.stack([stat_blocks(inputs["Wq"][l]) for l in range(L)])
    d["wk"] = np.stack([stat_blocks(inputs["Wk"][l]) for l in range(L)])
    d["wo"] = np.stack([stat_blocks(inputs["Wo"][l]) for l in range(L)])
    d["wi"] = np.stack([stat_blocks(inputs["Wi"][l]) for l in range(L)])
    d["wv"] = np.ascontiguousarray(inputs["Wv"][:L])
    d["wout"] = np.ascontiguousarray(inputs["Wout"][:L])
    d["pw"] = stat_blocks(inputs["pool_W"])

    def bias_t(b):   # [L, H or FF] -> [L, 128, chunks]
        return np.ascontiguousarray(
            b[:L].reshape(L, -1, 128).transpose(0, 2, 1))

    d["bq"] = bias_t(inputs["bq"])
    d["bk"] = bias_t(inputs["bk"])
    d["bo"] = bias_t(inputs["bo"])
    d["bi"] = bias_t(inputs["bi"])
    d["bout"] = bias_t(inputs["bout"])
    d["pb"] = np.ascontiguousarray(inputs["pool_b"].reshape(-1, 128).T)
    d["wemb"] = inputs["word_emb"]
    d["temb"] = inputs["type_emb"]
    d["pemb"] = inputs["pos_emb"]
    return d


def kernel(**inputs):
    L = 12
    nc = _get_nc(L)
    shared = _prep_weights(inputs, L)
    shared = {k: np.asarray(v, dtype=np.float32) for k, v in shared.items()}
    in_maps = []
    for b in range(N_CORES):
        m = dict(shared)
        m["ids"] = np.ascontiguousarray(inputs["input_ids"][b]).astype(np.int32)
        m["tts"] = np.ascontiguousarray(inputs["token_type_ids"][b]).astype(np.int32)
        in_maps.append(m)
    res = run_bass_kernel_spmd(nc, in_maps, core_ids=list(range(N_CORES)))
    x = np.stack([res.results[b]["x_out"] for b in range(N_CORES)])
    pooled = np.stack([res.results[b]["pool_out"] for b in range(N_CORES)])
    return x.astype(np.float32), pooled.astype(np.float32)
